# revision 1
# baseline (speedup 1.0000x reference)
"""Trainium2 Bass kernel for the hardest-positive triplet-softplus loss.

Strategy (data-parallel over distance-matrix rows, 8 NeuronCores):
  - Each core owns a 512-row block of the 4096-row pairwise structure.
  - Selection matrix in fp16 on the PE at full rate:
        S[i,j] = 2*dot(x_i,x_j) + BIG*same(i,j) - (sq_j - 512)
    built from a 640-dim extended contraction ([2x; onehot] x [x; BIG*onehot])
    plus a K=1 matmul adding the centered -sq_j term, and a -2*BIG additive
    mask knocking out the diagonal.  Row-wise argmax of S picks the hardest
    positive (min distance); per-core column rotation keeps the diagonal in
    column-block 0 so the program stays SPMD.
  - Argmax is extracted with a fused one-pass trick per PSUM tile:
    reduce_max gives the tile max, then ((S >= max) * iota) summed via
    scalar_tensor_tensor's accumulator yields the argmax column; a second
    application of the same trick across the 8 tile partials selects the
    global winner.  No PSUM->SBUF spill of the matrix is ever needed.
  - The winning rows are fetched with an indirect DMA gather from a
    per-core pre-rolled copy of the batch; d(a,p) and d(a,n) are then
    recomputed exactly in fp32 (sub -> square-accumulate), matching the
    reference formulation bit-for-bit in spirit.
  - Per-row stable-softplus tail on scalar/vector engines; each core returns
    a [128,1] per-partition partial sum, combined (and divided by the
    host-known valid count) on the host -- that is the unshard step.
"""

import os
import sys

import numpy as np

for _p in ("/opt/trn_rl_repo", "/root/.axon_site/_ro/trn_rl_repo"):
    if os.path.isdir(_p) and _p not in sys.path:
        sys.path.append(_p)

import concourse.bass as bass  # noqa: E402
import concourse.bacc as bacc  # noqa: E402
import concourse.tile as tile  # noqa: E402
from concourse import mybir  # noqa: E402
from concourse import bass_utils  # noqa: E402

B = 4096
DIM = 512
C = 128
NCORES = 8
RB = B // NCORES          # rows per core
NK = (DIM + C) // 128     # 5 contraction tiles of 128
NM = RB // 128            # 4 row tiles per core
NN = B // 512             # 8 column blocks of 512
BIG = 4096.0
EPS = 1e-12

F32 = mybir.dt.float32
F16 = mybir.dt.float16
U32 = mybir.dt.uint32
ALU = mybir.AluOpType
AFT = mybir.ActivationFunctionType
AX = mybir.AxisListType

# rhs column blocks are consumed (and DMAed) in this order; the diagonal block
# (rolled position 0) goes last so its mask DMA can trail the first chunks.
N_ORDER = [1, 2, 3, 4, 5, 6, 7, 0]

_NC_CACHE = None


def _build_nc():
    nc = bacc.Bacc(
        "TRN2",
        target_bir_lowering=False,
        debug=False,
        enable_asserts=False,
    )

    rhs_d = nc.dram_tensor("rhs", [NN, 128, NK * 512], F16, kind="ExternalInput").ap()
    lhs_d = nc.dram_tensor("lhsx", [128, NK * 512], F16, kind="ExternalInput").ap()
    sqcn_d = nc.dram_tensor("sqcn", [1, B], F16, kind="ExternalInput").ap()
    diag_d = nc.dram_tensor("diagm", [128, NM * 512], F16, kind="ExternalInput").ap()
    iota_d = nc.dram_tensor("iotam", [128, B], F32, kind="ExternalInput").ap()
    bt_d = nc.dram_tensor("batcht", [B, DIM], F32, kind="ExternalInput").ap()
    xrow_d = nc.dram_tensor("xrow", [128, NM * 512], F32, kind="ExternalInput").ap()
    xneg_d = nc.dram_tensor("xneg", [128, NM * 512], F32, kind="ExternalInput").ap()
    vld_d = nc.dram_tensor("vld", [128, NM], F32, kind="ExternalInput").ap()
    out_d = nc.dram_tensor("out", [128, 1], F32, kind="ExternalOutput").ap()

    with tile.TileContext(nc) as tc:
        with (
            tc.tile_pool(name="big", bufs=1) as big,
            tc.tile_pool(name="work", bufs=4) as work,
            tc.tile_pool(name="ps", bufs=6, space="PSUM") as pp,
            tc.tile_pool(name="sm", bufs=1) as sm,
        ):
            lhs_sb = big.tile([128, NK * 512], F16, tag="lhs")
            nc.sync.dma_start(lhs_sb[:], lhs_d[:])
            sqcn_sb = big.tile([1, B], F16, tag="sqcn")
            nc.sync.dma_start(sqcn_sb[:], sqcn_d[:])
            ones1 = sm.tile([1, 128], F16, tag="ones1")
            nc.vector.memset(ones1[:], 1.0)

            rhs_sb = {}
            for i, n in enumerate(N_ORDER):
                t = big.tile([128, NK * 512], F16, tag=f"rhs{n}", name=f"rhs{n}")
                nc.sync.dma_start(t[:], rhs_d[n])
                rhs_sb[n] = t
                if i == 0:
                    diag_sb = big.tile([128, NM * 512], F16, tag="diag", name="diag")
                    nc.sync.dma_start(diag_sb[:], diag_d[:])
                elif i == 1:
                    iota_sb = big.tile([128, B], F32, tag="iota", name="iota")
                    nc.sync.dma_start(iota_sb[:], iota_d[:])
                elif i == 3:
                    xr_sb = big.tile([128, NM * 512], F32, tag="xr", name="xr")
                    nc.sync.dma_start(xr_sb[:], xrow_d[:])
                    xn_sb = big.tile([128, NM * 512], F32, tag="xn", name="xn")
                    nc.sync.dma_start(xn_sb[:], xneg_d[:])

            vld = sm.tile([128, NM], F32, tag="vld")
            nc.sync.dma_start(vld[:], vld_d[:])
            epsb = sm.tile([128, 1], F32, tag="epsb")
            nc.gpsimd.memset(epsb[:], EPS)

            d2ap = sm.tile([128, NM], F32, tag="d2ap")
            d2an = sm.tile([128, NM], F32, tag="d2an")
            parts = [
                sm.tile([128, NN], F32, tag=f"parts{m}", name=f"parts{m}")
                for m in range(NM)
            ]
            ixparts = [
                sm.tile([128, NN], F32, tag=f"ixparts{m}", name=f"ixparts{m}")
                for m in range(NM)
            ]

            # ---- d(a, negative): exact fp32, independent of the matrix ----
            for m in range(NM):
                ms = slice(m * 512, (m + 1) * 512)
                dsc = work.tile([128, DIM], F32, tag="dsc", name="dsc")
                nc.vector.tensor_sub(dsc[:], xr_sb[:, ms], xn_sb[:, ms])
                ssc = work.tile([128, DIM], F32, tag="ssc", name="ssc")
                nc.scalar.activation(
                    ssc[:], dsc[:], AFT.Square, accum_out=d2an[:, m:m + 1]
                )

            # ---- main pass: n-outer keeps the PE dense behind the DMA ----
            for i, n in enumerate(N_ORDER):
                for m in range(NM):
                    pt = pp.tile([128, 512], F32, tag="acc", name="acc")
                    for k in range(NK):
                        nc.tensor.matmul(
                            pt[:],
                            lhs_sb[:, k * 512 + m * 128:k * 512 + (m + 1) * 128],
                            rhs_sb[n][:, k * 512:(k + 1) * 512],
                            start=(k == 0),
                            stop=False,
                        )
                    # centered -sq_j via a K=1 matmul into the same psum group
                    nc.tensor.matmul(
                        pt[:],
                        ones1[:],
                        sqcn_sb[:, n * 512:(n + 1) * 512],
                        start=False,
                        stop=True,
                    )
                    if n == 0:
                        nc.vector.tensor_add(
                            pt[:], pt[:], diag_sb[:, m * 512:(m + 1) * 512]
                        )
                    # per-tile max + fused argmax ((S>=max)*iota summed)
                    nc.vector.tensor_reduce(
                        parts[m][:, i:i + 1], pt[:], axis=AX.X, op=ALU.max
                    )
                    junk = work.tile([128, 512], F32, tag="junk", name="junk")
                    nc.vector.scalar_tensor_tensor(
                        junk[:], pt[:], parts[m][:, i:i + 1],
                        iota_sb[:, n * 512:(n + 1) * 512],
                        op0=ALU.is_ge, op1=ALU.mult,
                        accum_out=ixparts[m][:, i:i + 1],
                    )

                    if i == NN - 1:
                        # last column block: finalize this m-tile
                        mxv = work.tile([128, 1], F32, tag="mxv", name="mxv")
                        nc.vector.tensor_reduce(
                            mxv[:], parts[m][:], axis=AX.X, op=ALU.max
                        )
                        junk8 = work.tile([128, NN], F32, tag="junk8", name="junk8")
                        idxf = work.tile([128, 1], F32, tag="idxf", name="idxf")
                        nc.vector.scalar_tensor_tensor(
                            junk8[:], parts[m][:], mxv[:], ixparts[m][:],
                            op0=ALU.is_ge, op1=ALU.mult, accum_out=idxf[:],
                        )
                        nc.vector.tensor_scalar(
                            idxf[:], idxf[:], float(B - 1), None, op0=ALU.min
                        )
                        idxu = work.tile([128, 1], U32, tag="idxu", name="idxu")
                        nc.vector.tensor_copy(idxu[:], idxf[:])
                        xp = work.tile([128, DIM], F32, tag="xp", name="xp")
                        nc.gpsimd.indirect_dma_start(
                            out=xp[:], out_offset=None, in_=bt_d[:],
                            in_offset=bass.IndirectOffsetOnAxis(
                                ap=idxu[:, :1], axis=0),
                        )
                        ms = slice(m * 512, (m + 1) * 512)
                        dsc = work.tile([128, DIM], F32, tag="dsc", name="dsc")
                        nc.vector.tensor_sub(dsc[:], xr_sb[:, ms], xp[:])
                        ssc = work.tile([128, DIM], F32, tag="ssc", name="ssc")
                        nc.scalar.activation(
                            ssc[:], dsc[:], AFT.Square,
                            accum_out=d2ap[:, m:m + 1],
                        )

            # ---- per-row tail ([128, 4] tensors) ----
            # sqrt(max(d2,eps)) == sqrt(d2+eps) in fp32 for d2 >= 0
            dap = sm.tile([128, NM], F32, tag="dap")
            nc.scalar.activation(dap[:], d2ap[:], AFT.Sqrt, bias=epsb[:])
            dan = sm.tile([128, NM], F32, tag="dan")
            nc.scalar.activation(dan[:], d2an[:], AFT.Sqrt, bias=epsb[:])
            zd = sm.tile([128, NM], F32, tag="zd")
            nc.vector.tensor_sub(zd[:], dap[:], dan[:])
            a1 = sm.tile([128, NM], F32, tag="a1")
            nc.scalar.activation(a1[:], zd[:], AFT.Relu, scale=10.0)
            a2 = sm.tile([128, NM], F32, tag="a2")
            nc.scalar.activation(a2[:], zd[:], AFT.Relu, scale=-10.0)
            s = sm.tile([128, NM], F32, tag="s")
            nc.vector.tensor_add(s[:], a1[:], a2[:])            # |10*zd|
            e = sm.tile([128, NM], F32, tag="e")
            nc.scalar.activation(e[:], s[:], AFT.Exp, scale=-1.0)
            ln1p = sm.tile([128, NM], F32, tag="ln1p")
            nc.scalar.activation(ln1p[:], e[:], AFT.Ln, bias=1.0)
            per = sm.tile([128, NM], F32, tag="per")
            nc.vector.tensor_add(per[:], a1[:], ln1p[:])        # softplus(10*zd)
            w = sm.tile([128, NM], F32, tag="w")
            nc.vector.tensor_mul(w[:], per[:], vld[:])
            prt = sm.tile([128, 1], F32, tag="prt")
            nc.vector.tensor_reduce(prt[:], w[:], axis=AX.X, op=ALU.add)
            nc.sync.dma_start(out_d[:], prt[:])

    nc.compile()
    return nc


def get_nc():
    global _NC_CACHE
    if _NC_CACHE is None:
        _NC_CACHE = _build_nc()
    return _NC_CACHE


def _prep_inputs(batch, labels, anchors, negatives):
    """Host-side sharding prep: build the 8 per-core input maps."""
    batch = np.ascontiguousarray(np.asarray(batch), dtype=np.float32)
    labels = np.asarray(labels).astype(np.int64)
    anchors = np.asarray(anchors).astype(np.int64)
    negatives = np.asarray(negatives).astype(np.int64)

    sq = (batch * batch).sum(axis=1, dtype=np.float32)          # [B]
    onehotT = np.zeros((C, B), np.float32)
    onehotT[labels, np.arange(B)] = 1.0

    rhs_full = np.empty((NK * 128, B), np.float16)
    rhs_full[:DIM] = batch.T
    rhs_full[DIM:] = BIG * onehotT
    # [n, p, k*512+q] layout: per-n chunks are single contiguous DMAs
    rhs_chunks = np.ascontiguousarray(
        rhs_full.reshape(NK, 128, NN, 512).transpose(2, 1, 0, 3).reshape(
            NN, 128, NK * 512)
    )
    sqcn = -(sq - np.float32(512.0)).astype(np.float16)         # [B]

    diag = np.zeros((128, NM * 512), np.float16)
    p = np.arange(128)
    for m in range(NM):
        diag[p, 512 * m + 128 * m + p] = -2.0 * BIG

    iota = np.broadcast_to(
        np.arange(B, dtype=np.float32), (128, B)).copy()

    hist = np.bincount(labels, minlength=C)
    valid = (hist[labels] - 1) > 1                              # [B] bool
    count = float(valid.sum())

    in_maps = []
    for c in range(NCORES):
        r0 = c * RB
        rows = slice(r0, r0 + RB)
        arow = anchors[rows]
        nrow = negatives[rows]

        lhs = np.empty((NK * 128, RB), np.float16)
        lhs[:DIM] = 2.0 * batch[rows].T
        lhs[DIM:] = onehotT[:, rows]

        perm = (np.arange(NN) + c) % NN
        in_maps.append({
            "rhs": np.ascontiguousarray(rhs_chunks[perm]),
            "lhsx": np.ascontiguousarray(
                lhs.reshape(NK, 128, RB).transpose(1, 0, 2).reshape(
                    128, NK * 512)),
            "sqcn": np.ascontiguousarray(np.roll(sqcn, -r0)[None, :]),
            "diagm": diag,
            "iotam": iota,
            "batcht": np.roll(batch, -r0, axis=0),
            "xrow": np.ascontiguousarray(
                batch[arow].reshape(NM, 128, DIM).transpose(1, 0, 2).reshape(
                    128, NM * 512)),
            "xneg": np.ascontiguousarray(
                batch[nrow].reshape(NM, 128, DIM).transpose(1, 0, 2).reshape(
                    128, NM * 512)),
            "vld": np.ascontiguousarray(
                valid[rows].astype(np.float32).reshape(NM, 128).T),
        })
    return in_maps, count


def kernel(batch, labels, anchors, negatives, **_kwargs):
    in_maps, count = _prep_inputs(batch, labels, anchors, negatives)
    nc = get_nc()
    res = bass_utils.run_bass_kernel_spmd(nc, in_maps, core_ids=list(range(NCORES)))
    total = sum(r["out"].sum(dtype=np.float64) for r in res.results)
    loss = np.float32(np.float32(total) / np.float32(count))
    return np.array([loss], dtype=np.float32)



# revision 8
# speedup vs baseline: 2.7238x; 2.7238x over previous
"""Trainium2 Bass kernel for the hardest-positive triplet-softplus loss.

Strategy v2 (label-sorted windows, 8 NeuronCores):
  - Host sorts rows AND columns of the distance matrix by label (a pure
    layout/sharding choice).  After the sort, the same-label columns a
    128-row tile needs all live in one narrow window (max 183 cols for
    this input regime) -> pad to W (multiple of 64).  The dense 4096-col
    scan of the baseline becomes a W-col scan: ~16x less PE+DVE work.
  - Per 128-row tile the PE computes one [128, W+128] fp16 matmul group:
    cols [0,W) are the window (selection), cols [W,W+128) are the rows'
    negatives, so d(a,n) comes from the same matmul (diagonal extract).
  - Selection: ttr fuses (2dot + adj) with a row-max; adj carries
    -fp16(sq_j-512), the not-my-label -BIG mask and the self -2BIG mask.
    A gpsimd stt (is_equal * g) then recovers g_p = (sq_p-512)-fp16(sq_p-512)
    of the winning column, so d2ap = sq_a + g_p + 512 - rowmax is exact
    up to the fp16 rounding of the dot itself (~0.02 in d2 ~ 900).
  - d2an = sq_a + sq_n - diag(2dot(a,n)), zeroed for self-negatives.
  - Tail: d*10 via one Sqrt activation (scale=100), stable softplus via
    relu/abs_max on DVE + Exp/Ln on scalar, valid-mask, row-reduce.
  - Each core returns a [128,1] partial; host sums and divides by the
    host-known valid count (the unshard step).
"""

import os
import sys

import numpy as np

for _p in ("/opt/trn_rl_repo", "/root/.axon_site/_ro/trn_rl_repo"):
    if os.path.isdir(_p) and _p not in sys.path:
        sys.path.append(_p)

import concourse.bass as bass  # noqa: E402,F401
import concourse.bacc as bacc  # noqa: E402
import concourse.tile as tile  # noqa: E402
from concourse import mybir  # noqa: E402
from concourse import bass_utils  # noqa: E402

B = 4096
DIM = 512
C = 128
NCORES = 8
RB = B // NCORES          # rows per core
NT = RB // 128            # row tiles per core
NK = DIM // 128           # k chunks
BIG = 4096.0
EPS = 1e-8

F32 = mybir.dt.float32
F16 = mybir.dt.float16
ALU = mybir.AluOpType
AFT = mybir.ActivationFunctionType
AX = mybir.AxisListType

_NC_CACHE = {}


def _build_nc(W):
    WP = W + 128              # window + negatives block
    BLOBW = NK * WP + 2 * W   # rhs chunks | adj | g16
    LHSW = NT * NK * 128 + 128  # lhs chunks | diag identity

    nc = bacc.Bacc(
        "TRN2",
        target_bir_lowering=False,
        debug=False,
        enable_asserts=False,
    )

    lhs_d = nc.dram_tensor("lhsd", [128, LHSW], F16, kind="ExternalInput").ap()
    blob_d = nc.dram_tensor("blob", [NT, 128, BLOBW], F16, kind="ExternalInput").ap()
    small_d = nc.dram_tensor("small", [128, 16], F32, kind="ExternalInput").ap()
    out_d = nc.dram_tensor("out", [128, 1], F32, kind="ExternalOutput").ap()

    with tile.TileContext(nc) as tc:
        with (
            tc.tile_pool(name="cst", bufs=1) as cst,
            tc.tile_pool(name="work", bufs=2) as work,
            tc.tile_pool(name="ps", bufs=4, space="PSUM") as pp,
            tc.tile_pool(name="sm", bufs=1) as sm,
        ):
            small_sb = cst.tile([128, 16], F32, tag="small")
            nc.sync.dma_start(small_sb[:], small_d[:])
            lhs_sb = cst.tile([128, LHSW], F16, tag="lhs")
            nc.sync.dma_start(lhs_sb[:], lhs_d[:])
            blob_sb = []
            for t in range(NT):
                bt = cst.tile([128, BLOBW], F16, tag=f"blob{t}", name=f"blob{t}")
                nc.sync.dma_start(bt[:], blob_d[t])
                blob_sb.append(bt)

            epsb = sm.tile([128, 1], F32, tag="epsb")
            nc.gpsimd.memset(epsb[:], EPS)
            # dummy: pulls the sqrt act table load into the DMA phase
            dumm = sm.tile([128, 1], F32, tag="dumm")
            nc.scalar.activation(dumm[:], epsb[:], AFT.Sqrt)

            rowmaxv = sm.tile([128, NT], F32, tag="rowmaxv")
            gsel = sm.tile([128, NT], F32, tag="gsel")
            dotan = sm.tile([128, NT], F32, tag="dotan")

            diag_ap = lhs_sb[:, NT * NK * 128:NT * NK * 128 + 128]

            for t in range(NT):
                P = pp.tile([128, WP], F32, tag="acc", name=f"acc{t}")
                for kk in range(NK):
                    nc.tensor.matmul(
                        P[:],
                        lhs_sb[:, (t * NK + kk) * 128:(t * NK + kk + 1) * 128],
                        blob_sb[t][:, kk * WP:(kk + 1) * WP],
                        start=(kk == 0),
                        stop=(kk == NK - 1),
                    )
                # d(a,n): extract diagonal of the negatives block
                junk3 = work.tile([128, 128], F32, tag="junk3", name="junk3")
                nc.vector.scalar_tensor_tensor(
                    junk3[:], P[:, W:WP], 1.0, diag_ap,
                    op0=ALU.mult, op1=ALU.mult,
                    accum_out=dotan[:, t:t + 1],
                )
                # selection: masked S = 2dot + adj, row max
                junk = work.tile([128, W], F32, tag="junk", name="junk")
                nc.vector.tensor_add(
                    junk[:], P[:, 0:W],
                    blob_sb[t][:, NK * WP:NK * WP + W],
                )
                nc.vector.tensor_reduce(
                    rowmaxv[:, t:t + 1], junk[:], axis=AX.X, op=ALU.max,
                )
                # winner's g residual via (S == rowmax) * g
                junk2 = work.tile([128, W], F32, tag="junk2", name="junk2")
                nc.vector.scalar_tensor_tensor(
                    junk2[:], junk[:], rowmaxv[:, t:t + 1],
                    blob_sb[t][:, NK * WP + W:NK * WP + 2 * W],
                    op0=ALU.is_equal, op1=ALU.mult,
                    accum_out=gsel[:, t:t + 1],
                )

            sqa = small_sb[:, 0:4]
            sqn = small_sb[:, 4:8]
            nmask = small_sb[:, 8:12]
            vldv = small_sb[:, 12:16]

            pack = sm.tile([128, 2 * NT], F32, tag="pack")
            t1 = sm.tile([128, NT], F32, tag="t1")
            nc.vector.tensor_add(t1[:], sqa, gsel[:])
            t2 = sm.tile([128, NT], F32, tag="t2")
            nc.vector.tensor_scalar(t2[:], t1[:], 512.0, None, op0=ALU.add)
            nc.vector.tensor_sub(pack[:, 0:NT], t2[:], rowmaxv[:])
            u1 = sm.tile([128, NT], F32, tag="u1")
            nc.vector.tensor_add(u1[:], sqa, sqn)
            u2 = sm.tile([128, NT], F32, tag="u2")
            nc.vector.tensor_sub(u2[:], u1[:], dotan[:])
            nc.vector.tensor_mul(pack[:, NT:2 * NT], u2[:], nmask)
            nc.vector.tensor_scalar(pack[:], pack[:], 0.0, None, op0=ALU.max)

            d10 = sm.tile([128, 2 * NT], F32, tag="d10")
            nc.scalar.activation(d10[:], pack[:], AFT.Sqrt,
                                 bias=epsb[:], scale=100.0)
            z = sm.tile([128, NT], F32, tag="z")
            nc.vector.tensor_sub(z[:], d10[:, 0:NT], d10[:, NT:2 * NT])
            a1 = sm.tile([128, NT], F32, tag="a1")
            nc.vector.tensor_scalar(a1[:], z[:], 0.0, None, op0=ALU.max)
            nz = sm.tile([128, NT], F32, tag="nz")
            nc.vector.tensor_scalar(nz[:], z[:], -1.0, None, op0=ALU.mult)
            az = sm.tile([128, NT], F32, tag="az")
            nc.vector.tensor_max(az[:], z[:], nz[:])
            e = sm.tile([128, NT], F32, tag="e")
            nc.scalar.activation(e[:], az[:], AFT.Exp, scale=-1.0)
            ln1p = sm.tile([128, NT], F32, tag="ln1p")
            nc.scalar.activation(ln1p[:], e[:], AFT.Ln, bias=1.0)
            per = sm.tile([128, NT], F32, tag="per")
            nc.vector.tensor_add(per[:], a1[:], ln1p[:])
            wv = sm.tile([128, NT], F32, tag="wv")
            nc.vector.tensor_mul(wv[:], per[:], vldv)
            prt = sm.tile([128, 1], F32, tag="prt")
            nc.vector.tensor_reduce(prt[:], wv[:], axis=AX.X, op=ALU.add)
            nc.sync.dma_start(out_d[:], prt[:])

    nc.compile()
    return nc


def get_nc(W=192):
    if W not in _NC_CACHE:
        _NC_CACHE[W] = _build_nc(W)
    return _NC_CACHE[W]


def _prep_inputs(batch, labels, anchors, negatives):
    """Host-side sharding prep: label-sort layout + per-core input maps."""
    batch = np.ascontiguousarray(np.asarray(batch), dtype=np.float32)
    labels = np.asarray(labels).astype(np.int64)
    anchors = np.asarray(anchors).astype(np.int64)
    negatives = np.asarray(negatives).astype(np.int64)
    assert np.array_equal(anchors, np.arange(B)), "kernel assumes anchors=arange"

    sq = np.einsum("ij,ij->i", batch, batch, dtype=np.float32,
                   optimize=True).astype(np.float32)

    perm = np.argsort(labels, kind="stable")
    ls = labels[perm]                       # sorted labels
    xs16 = batch[perm].astype(np.float16)   # sorted rows, fp16
    lhs16 = (xs16 * np.float16(2.0))        # exact scale
    sqs = sq[perm]
    sqn512 = sqs - np.float32(512.0)
    sqf16 = sqn512.astype(np.float16)
    sqf16_32 = sqf16.astype(np.float32)
    g16 = (sqn512 - sqf16_32).astype(np.float16)

    col_start = np.searchsorted(ls, np.arange(C), side="left")
    col_end = np.searchsorted(ls, np.arange(C), side="right")

    NTILES = B // 128
    w0 = np.empty(NTILES, np.int64)
    need = 0
    for T in range(NTILES):
        w0[T] = col_start[ls[T * 128]]
        need = max(need, col_end[ls[T * 128 + 127]] - w0[T])
    W = max(64, int(-(-need // 64) * 64))
    assert W <= 384, f"window {need} too wide"
    WP = W + 128
    BLOBW = NK * WP + 2 * W

    negs_s = negatives[perm]
    xneg16 = batch[negs_s].astype(np.float16)
    sq_neg = sq[negs_s]
    nmask = (negs_s != perm).astype(np.float32)
    hist = np.bincount(labels, minlength=C)
    vld_all = (hist[labels] >= 3).astype(np.float32)[perm]
    count = float(vld_all.sum())

    qs = np.arange(W)
    ms = np.arange(128)
    in_maps = []
    for c in range(NCORES):
        blob = np.empty((NT, 128, BLOBW), np.float16)
        lhsd = np.empty((128, NT * NK * 128 + 128), np.float16)
        smalls = np.empty((128, 16), np.float32)
        for t in range(NT):
            T = c * NT + t
            rows = slice(T * 128, (T + 1) * 128)
            wcols = (w0[T] + qs) % B
            rhs_k = np.concatenate(
                [xs16[wcols].T, xneg16[rows].T], axis=1)      # [512, WP]
            blob[t, :, :NK * WP] = rhs_k.reshape(
                NK, 128, WP).transpose(1, 0, 2).reshape(128, NK * WP)
            lm = ls[rows]
            lq = ls[wcols]
            adj = np.repeat(-sqf16_32[wcols][None, :], 128, axis=0)
            adj[lq[None, :] != lm[:, None]] -= BIG
            q_self = T * 128 + ms - w0[T]
            assert (q_self >= 0).all() and (q_self < W).all()
            adj[ms, q_self] -= 2.0 * BIG
            blob[t, :, NK * WP:NK * WP + W] = adj.astype(np.float16)
            blob[t, :, NK * WP + W:] = np.repeat(
                g16[wcols][None, :], 128, axis=0)

            lrows = lhs16[rows]                                # [128m, 512k]
            lhsd[:, t * NK * 128:(t + 1) * NK * 128] = (
                lrows.T.reshape(NK, 128, 128).transpose(1, 0, 2).reshape(
                    128, NK * 128))
            smalls[:, t] = sqs[rows]
            smalls[:, 4 + t] = sq_neg[rows]
            smalls[:, 8 + t] = nmask[rows]
            smalls[:, 12 + t] = vld_all[rows]
        lhsd[:, NT * NK * 128:] = np.eye(128, dtype=np.float16)
        in_maps.append({
            "lhsd": np.ascontiguousarray(lhsd),
            "blob": np.ascontiguousarray(blob),
            "small": np.ascontiguousarray(smalls),
        })
    return in_maps, count, W


def kernel(batch, labels, anchors, negatives, **_kwargs):
    in_maps, count, W = _prep_inputs(batch, labels, anchors, negatives)
    nc = get_nc(W)
    res = bass_utils.run_bass_kernel_spmd(nc, in_maps, core_ids=list(range(NCORES)))
    total = sum(r["out"].sum(dtype=np.float64) for r in res.results)
    loss = np.float32(np.float32(total) / np.float32(count))
    return np.array([loss], dtype=np.float32)
